# revision 10
# baseline (speedup 1.0000x reference)
"""Dilated attention Trainium2 kernel.

Problem: for each (batch, segment) pair, and each dilation rate r in {1,2,4,8}:
  q = Q_seg[::r], k = K_seg[::r], v = V_seg[::r]
  out_seg[::r] += softmax(q @ k.T) @ v        (no 1/sqrt(d) scaling)

Sharding: B=2 x n_seg=4 = 8 independent (batch, segment) pairs -> one per core.

Per-core kernel structure:
  - cast Q,K to fp16 in DRAM scratch, xbar-transpose-DMA into SBUF as [d, l]
    (PE contracts over the partition dim, so scores need d on partitions).
  - rate-r views are free-dim stride-r slices of the transposed tensors.
  - scores S[q,k] in PSUM fp32; row-max (negated) on DVE; exp+rowsum fused in
    one ScalarE activation (bias=-max, accum_out=rowsum) -> P fp16 in SBUF.
  - P tiles transposed via PE (identity matmul) -> P^T fp16, then PV matmuls
    with V fp16 (cast during DMA load) accumulate O in PSUM fp32.
  - O normalized by 1/rowsum on eviction. Rates 8,4,2 write to DRAM scratch;
    rate 1 runs last and pulls scratch rows into its output tile with
    partition-strided accumulate-DMAs (SWDGE CCE add), then stores once.
"""

import sys

if "/opt/trn_rl_repo" not in sys.path:
    sys.path.insert(0, "/opt/trn_rl_repo")

import numpy as np

import concourse.bass as bass
import concourse.mybir as mybir
from concourse import tile
from concourse.masks import make_identity
from concourse.tile_rust import add_dep_helper
from concourse.bass_utils import run_bass_kernel_spmd

SEG_LEN = 2048
D = 1024
P = 128
NDCH = D // P  # 8 d-chunks of 128
RATES = (8, 4, 2, 1)  # rate 1 last: it owns the final combine + store
F16 = mybir.dt.float16
F32 = mybir.dt.float32

_ws_ctr = [0]


def _split_multi_waits(nc):
    """walrus in this env accepts only ONE sync-wait per instruction; move
    extras onto same-engine NoOps inserted right before the instruction."""
    for f in nc.m.functions:
        for b in f.blocks:
            out, changed = [], False
            for inst in b.instructions:
                si = inst.sync_info
                if si is not None and si.on_wait and len(si.on_wait) > 1:
                    waits = list(si.on_wait)
                    for w in waits[:-1]:
                        nop = mybir.InstNoOp(
                            name=f"waitsplit_{_ws_ctr[0]}", ins=[], outs=[]
                        )
                        _ws_ctr[0] += 1
                        nop.engine = inst.engine
                        nop.sync_info = mybir.SyncInfo(on_wait=[w], on_update=[])
                        out.append(nop)
                    si.on_wait = [waits[-1]]
                    changed = True
                out.append(inst)
            if changed:
                b.instructions = out


_LDW_PATCHED = [False]


def _enable_ldw_opt():
    """walrus is invoked with --enable-ldw-opt=false by default; turning it on
    dedupes LDWEIGHTS for consecutive matmuls sharing the stationary operand."""
    if _LDW_PATCHED[0]:
        return
    from concourse import bass_utils as bu

    orig = bu.run_command

    def patched(argv, **kw):
        argv = [
            "--enable-ldw-opt=true" if a == "--enable-ldw-opt=false" else a
            for a in argv
        ]
        return orig(argv, **kw)

    bu.run_command = patched
    _LDW_PATCHED[0] = True


def build_kernel():
    # note: --enable-ldw-opt=true crashes the device (NRT_EXEC_UNIT_UNRECOVERABLE)
    nc = bass.Bass()
    Q = nc.dram_tensor("Q", (SEG_LEN, D), F32, kind="ExternalInput")
    K = nc.dram_tensor("K", (SEG_LEN, D), F32, kind="ExternalInput")
    V = nc.dram_tensor("V", (SEG_LEN, D), F32, kind="ExternalInput")
    O = nc.dram_tensor("O", (SEG_LEN, D), F32, kind="ExternalOutput")

    with tile.TileContext(nc) as tc:
        with (
            tc.tile_pool(name="qkt", bufs=1) as qkt_pool,
            tc.tile_pool(name="vp", bufs=2) as v_pool,
            tc.tile_pool(name="pp", bufs=2) as p_pool,
            tc.tile_pool(name="pt", bufs=18) as pt_pool,
            tc.tile_pool(name="op", bufs=3) as o_pool,
            tc.tile_pool(name="st", bufs=8) as stat_pool,
            tc.tile_pool(name="misc", bufs=1) as misc_pool,
            tc.tile_pool(name="spsum", bufs=1, space="PSUM") as s_psum,
            tc.tile_pool(name="ptpsum", bufs=2, space="PSUM") as pt_psum,
            tc.tile_pool(name="opsum", bufs=1, space="PSUM") as o_psum,
            tc.tile_pool(name="dram", bufs=1, space="DRAM") as dram_pool,
        ):
            ident = misc_pool.tile([P, P], F16)
            make_identity(nc, ident[:])

            # ---- cast Q,K to fp16 in DRAM, then xbar-transpose into SBUF ----
            # chunked by d so the first matmuls can start after ~2 chunks
            # (separate tiles per chunk => per-chunk dependency granularity)
            QT = [
                qkt_pool.tile([P, SEG_LEN], F16, tag=f"QT{c}", name=f"QT{c}")
                for c in range(NDCH)
            ]
            KT = [
                qkt_pool.tile([P, SEG_LEN], F16, tag=f"KT{c}", name=f"KT{c}")
                for c in range(NDCH)
            ]
            qh16 = dram_pool.tile([SEG_LEN, D], F16, tag="qh16")
            kh16 = dram_pool.tile([SEG_LEN, D], F16, tag="kh16")
            for c in range(NDCH):
                cs = slice(c * P, (c + 1) * P)
                nc.gpsimd.dma_start(qh16[:, cs], Q[:, cs])  # fp32->fp16 cast
                nc.sync.dma_start(QT[c][:], qh16[:, cs], transpose=True)
                nc.gpsimd.dma_start(kh16[:, cs], K[:, cs])
                nc.sync.dma_start(KT[c][:], kh16[:, cs], transpose=True)

            # rate scratch: normalized outputs of rates 8,4,2 (rows = q index)
            scratch = {
                r: dram_pool.tile(
                    [SEG_LEN // r, D], F32, tag=f"sc{r}", name=f"sc{r}"
                )
                for r in RATES
                if r > 1
            }
            rate_barrier = {}  # rate -> nop inst all scratch writes feed into

            for r in RATES:
                L = SEG_LEN // r
                n_kt = L // P  # k-tiles of 128
                # V rows for this rate, cast to fp16 on load: [128, n_kt, 1024]
                Vt = v_pool.tile([P, 16, D], F16, tag="V")
                for kt in range(n_kt):
                    row0 = kt * P * r
                    nc.gpsimd.dma_start(
                        Vt[:, kt, :], V[row0 : row0 + P * r : r, :]
                    )

                write_insts = []
                for t in range(L // P):  # q-subtiles of 128 queries
                    S = s_psum.tile([P, SEG_LEN], F32, tag="S", name="S")[:, :L]
                    q0 = t * P * r
                    nblk = (L + 511) // 512
                    for d in range(NDCH):
                        lhsT = QT[d][:, q0 : q0 + P * r : r]
                        for n0 in range(0, L, 512):
                            n1 = min(L, n0 + 512)
                            nc.tensor.matmul(
                                S[:, n0:n1],
                                lhsT,
                                KT[d][:, n0 * r : n1 * r : r],
                                start=(d == 0),
                                stop=(d == NDCH - 1),
                            )
                    # per-block partial maxes overlap the tail of the score
                    # matmuls; the serial softmax tail is just combine + exp
                    partmax = stat_pool.tile([P, 4], F32, tag="partmax")
                    for b in range(nblk):
                        n0 = b * 512
                        n1 = min(L, n0 + 512)
                        nc.vector.tensor_reduce(
                            partmax[:, b : b + 1], S[:, n0:n1],
                            mybir.AxisListType.X, mybir.AluOpType.max,
                        )
                    negmax = stat_pool.tile([P, 1], F32, tag="negmax")
                    nc.vector.tensor_reduce(
                        negmax[:], partmax[:, :nblk], mybir.AxisListType.X,
                        mybir.AluOpType.max, negate=True,
                    )
                    Pt = p_pool.tile([P, SEG_LEN], F16, tag="P", name="Pt")[:, :L]
                    rowsum = stat_pool.tile([P, 1], F32, tag="rowsum")
                    if L >= 1024:
                        # split exp so P transposes can start after half 1
                        Lh = L // 2
                        rowsum2 = stat_pool.tile([P, 1], F32, tag="rowsum2")
                        nc.scalar.activation(
                            Pt[:, :Lh], S[:, :Lh],
                            mybir.ActivationFunctionType.Exp,
                            bias=negmax[:], scale=1.0, accum_out=rowsum[:],
                        )
                        nc.scalar.activation(
                            Pt[:, Lh:], S[:, Lh:],
                            mybir.ActivationFunctionType.Exp,
                            bias=negmax[:], scale=1.0, accum_out=rowsum2[:],
                        )
                        nc.vector.tensor_add(rowsum[:], rowsum[:], rowsum2[:])
                    else:
                        nc.scalar.activation(
                            Pt[:], S[:], mybir.ActivationFunctionType.Exp,
                            bias=negmax[:], scale=1.0, accum_out=rowsum[:],
                        )
                    rinv = stat_pool.tile([P, 1], F32, tag="rinv")
                    nc.vector.reciprocal(rinv[:], rowsum[:])

                    # transpose all P tiles first so PV never head-of-line
                    # blocks the PE on the PSUM->SBUF eviction
                    pts = []
                    for kt in range(n_kt):
                        ptp = pt_psum.tile([P, P], F16, tag="ptp")
                        nc.tensor.transpose(
                            ptp[:], Pt[:, kt * P : (kt + 1) * P], ident[:]
                        )
                        ptsb = pt_pool.tile([P, P], F16, tag="pts")
                        if kt % 2 == 0:
                            nc.scalar.copy(ptsb[:], ptp[:])
                        else:
                            nc.vector.tensor_copy(ptsb[:], ptp[:])
                        pts.append(ptsb)

                    Ops = o_psum.tile([P, D], F32, tag="O")
                    for kt in range(n_kt):
                        for n0 in (0, 512):
                            nc.tensor.matmul(
                                Ops[:, n0 : n0 + 512],
                                pts[kt][:],
                                Vt[:, kt, n0 : n0 + 512],
                                start=(kt == 0),
                                stop=(kt == n_kt - 1),
                            )
                    Osb = o_pool.tile([P, D], F32, tag="Osb")
                    nc.vector.tensor_scalar_mul(Osb[:], Ops[:], rinv[:])

                    if r > 1:
                        w = nc.sync.dma_start(
                            scratch[r][t * P : (t + 1) * P, :], Osb[:]
                        )
                        write_insts.append(w.ins)
                    else:
                        # combine: add scattered rows of rates 2,4,8, store once
                        for rr in (2, 4, 8):
                            nrow = P // rr
                            sq0 = t * P // rr
                            acc = nc.gpsimd.dma_start(
                                Osb[0:P:rr, :],
                                scratch[rr][sq0 : sq0 + nrow, :],
                                accum_op=mybir.AluOpType.add,
                            )
                            add_dep_helper(
                                acc.ins, rate_barrier[rr],
                                reason=f"rate{rr} scratch complete",
                            )
                        nc.sync.dma_start(O[t * P : (t + 1) * P, :], Osb[:])

                if r > 1:
                    bar = nc.gpsimd.nop()
                    for w in write_insts:
                        add_dep_helper(bar.ins, w, reason=f"rate{r} writes")
                    rate_barrier[r] = bar.ins

    _split_multi_waits(nc)
    return nc


_NC_CACHE = None


def kernel(Q, K, V):
    global _NC_CACHE
    Q = np.asarray(Q)
    K = np.asarray(K)
    V = np.asarray(V)
    B, S, Dm = Q.shape
    n_seg = S // SEG_LEN
    assert (B, S, Dm) == (2, 8192, 1024) and n_seg == 4

    if _NC_CACHE is None:
        _NC_CACHE = build_kernel()
    nc = _NC_CACHE

    in_maps = []
    for c in range(8):
        b, g = divmod(c, n_seg)
        sl = slice(g * SEG_LEN, (g + 1) * SEG_LEN)
        in_maps.append(
            {
                "Q": np.ascontiguousarray(Q[b, sl], dtype=np.float32),
                "K": np.ascontiguousarray(K[b, sl], dtype=np.float32),
                "V": np.ascontiguousarray(V[b, sl], dtype=np.float32),
            }
        )
    res = run_bass_kernel_spmd(nc, in_maps, core_ids=list(range(8)))
    out = np.empty((B, S, Dm), dtype=np.float32)
    for c in range(8):
        b, g = divmod(c, n_seg)
        out[b, g * SEG_LEN : (g + 1) * SEG_LEN, :] = res.results[c]["O"]
    return out


if __name__ == "__main__":
    rng = np.random.default_rng(0)
    Q = rng.standard_normal((2, 8192, 1024), dtype=np.float32)
    K = rng.standard_normal((2, 8192, 1024), dtype=np.float32)
    V = rng.standard_normal((2, 8192, 1024), dtype=np.float32)
    out = kernel(Q=Q, K=K, V=V)
    print("ran ok", out.shape, out.dtype, np.abs(out).mean())
